# revision 8
# baseline (speedup 1.0000x reference)
"""Trainium2 Bass kernel for nn_LowResNet (cluster-pooled stacked-GRU seq2seq).

Strategy:
  - Data-parallel over batch: B=64 -> 8 cores x 8 batches.
  - Host does the cheap pool/unpool einsums (<2% of FLOPs); the device runs
    the recurrent encoder (12 steps) + autoregressive decoder (12 steps),
    which is ~98% of the FLOPs and strictly sequential.
  - On-device layout: activations as [channels (partitions), B_local*C (free)].
    All GEMMs are weight-stationary fp32r matmuls with N=512 chunks.
  - Tiny-K inputs (x: K=2, y: K=1) are folded into per-timestep masked weight
    tiles of K=24/K=12 so every operand sits at partition base 0.
  - Decoder feedback y lives in a [13, 4096] SBUF tile written one row per
    step; the masked weight for step t selects row t.
"""
import os
import numpy as np

import concourse.bass as bass
import concourse.mybir as mybir
import concourse.tile as tile
import concourse.bass_utils as bass_utils
from concourse import bacc

F32 = mybir.dt.float32
F32R = mybir.dt.float32r
AF = mybir.ActivationFunctionType

# model dims (fixed by the problem spec)
B, F_IN, N, S = 64, 2, 4096, 12
C, H, OUT = 512, 256, 1
NCORES = 8
BL = B // NCORES          # local batch per core
M = BL * C                # 4096 GEMM rows per core
MC = 512                  # free-dim chunk (= one local batch)
NJ = M // MC              # 8 chunks
GO = 2 * H                # gate output channels (512)
TO = GO + H               # gates + cand output channels (768)

_PROG_CACHE: dict[int, object] = {}
LAST_EXEC_NS = None       # set when KBENCH_TRACE=1


def _build_program(horizon: int):
    nc = bacc.Bacc("TRN2", target_bir_lowering=False, debug=False,
                   enable_asserts=False, num_devices=NCORES)

    xct_d = nc.dram_tensor("xct", [2 * S, M], F32R, kind="ExternalInput").ap()
    enc0m_d = nc.dram_tensor("enc0m", [2 * S, S * TO], F32R, kind="ExternalInput").ap()
    enc0h_d = nc.dram_tensor("enc0h", [128, 2 * TO], F32R, kind="ExternalInput").ap()
    enc1_d = nc.dram_tensor("enc1", [128, 4 * TO], F32R, kind="ExternalInput").ap()
    dec0m_d = nc.dram_tensor("dec0m", [S, S * TO], F32R, kind="ExternalInput").ap()
    dec0h_d = nc.dram_tensor("dec0h", [128, 2 * TO], F32R, kind="ExternalInput").ap()
    dec1_d = nc.dram_tensor("dec1", [128, 4 * TO], F32R, kind="ExternalInput").ap()
    wproj_d = nc.dram_tensor("wproj", [128, 2], F32R, kind="ExternalInput").ap()
    bias_d = nc.dram_tensor("bias", [128, 24], F32, kind="ExternalInput").ap()
    y0_d = nc.dram_tensor("y0", [1, M], F32R, kind="ExternalInput").ap()
    zeros_d = nc.dram_tensor("zeros", [128, M], F32R, kind="ExternalInput").ap()
    y_d = nc.dram_tensor("y", [max(horizon, 1), M], F32R, kind="ExternalOutput").ap()

    with tile.TileContext(nc) as tc:
        with tc.tile_pool(name="consts", bufs=1) as consts, \
             tc.tile_pool(name="mask", bufs=1) as maskp, \
             tc.tile_pool(name="xy", bufs=1) as xyp, \
             tc.tile_pool(name="hp", bufs=1) as hp, \
             tc.tile_pool(name="ru", bufs=5) as rup, \
             tc.tile_pool(name="rh", bufs=3) as rhp, \
             tc.tile_pool(name="cp", bufs=4) as cpp, \
             tc.tile_pool(name="tmp", bufs=6) as tmpp, \
             tc.tile_pool(name="ystage", bufs=1) as ystage_pool, \
             tc.tile_pool(name="ps", bufs=6, space="PSUM") as psp, \
             tc.tile_pool(name="pj", bufs=2, space="PSUM") as pjp:

            # ---- constants in ----
            enc0h = consts.tile([128, 2, TO], F32R)
            enc1w = consts.tile([128, 4, TO], F32R)
            dec0h = consts.tile([128, 2, TO], F32R)
            dec1w = consts.tile([128, 4, TO], F32R)
            wproj = consts.tile([128, 2], F32R)
            biases = consts.tile([128, 24], F32)
            nc.sync.dma_start(enc0h[:], enc0h_d.rearrange("p (k o) -> p k o", k=2))
            nc.sync.dma_start(enc1w[:], enc1_d.rearrange("p (k o) -> p k o", k=4))
            nc.sync.dma_start(dec0h[:], dec0h_d.rearrange("p (k o) -> p k o", k=2))
            nc.sync.dma_start(dec1w[:], dec1_d.rearrange("p (k o) -> p k o", k=4))
            nc.sync.dma_start(wproj[:], wproj_d)
            nc.sync.dma_start(biases[:], bias_d)

            emask = maskp.tile([2 * S, S, TO], F32R, tag="mask")
            nc.sync.dma_start(emask[:], enc0m_d.rearrange("p (t o) -> p t o", t=S))
            xct = xyp.tile([2 * S, M], F32R, tag="xy")
            nc.sync.dma_start(xct[:], xct_d)

            # ---- state ----
            h00 = hp.tile([128, M], F32R)   # enc/dec layer0 h, channels 0:128
            h01 = hp.tile([128, M], F32R)   # layer0 h, channels 128:256
            h10 = hp.tile([128, M], F32R)   # layer1 h, channels 0:128
            h11 = hp.tile([128, M], F32R)
            for t_ in (h00, h01, h10, h11):
                nc.sync.dma_start(t_[:], zeros_d)
            h0 = (h00, h01)
            h1 = (h10, h11)

            def gru_layer(gk, ck, ht, bb):
                """One GRU layer update over all chunks.

                gk/ck: list of (lhsT_fn(ot_col_base), rhs_fn(j)) k-tiles for
                gates/cand; entries with rhs_fn=None consume this chunk's rh.
                ht: (h_lo, h_hi) state tiles, updated in place.
                bb: bias slot base.
                """
                for j in range(NJ):
                    ms = bass.ds(j * MC, MC)
                    gps = [psp.tile([128, MC], F32, tag="g", name=f"gps{o}")
                           for o in range(4)]
                    for ot in range(4):
                        nk = len(gk)
                        for ki, (wf, rf) in enumerate(gk):
                            nc.tensor.matmul(gps[ot][:], wf(ot * 128), rf(j),
                                             start=(ki == 0), stop=(ki == nk - 1))
                    ru = []
                    for ot in range(4):
                        dst = rup.tile([128, MC], F32, tag="ru", name=f"ru{ot}")
                        nc.scalar.activation(dst[:], gps[ot][:], AF.Sigmoid,
                                             bias=biases[:, bb + ot:bb + ot + 1], scale=1.0)
                        ru.append(dst)
                    rh = []
                    for kt in range(2):
                        dst = rhp.tile([128, MC], F32R, tag="rh", name=f"rh{kt}")
                        nc.vector.tensor_mul(dst[:], ru[kt][:],
                                             ht[kt][:, ms].bitcast(F32))
                        rh.append(dst)
                    cps = [psp.tile([128, MC], F32, tag="g", name=f"cps{o}")
                           for o in range(2)]
                    for ot in range(2):
                        nk = len(ck)
                        ki = 0
                        for (wf, rf) in ck:
                            rhs = rh[0][:] if rf == "rh0" else \
                                  rh[1][:] if rf == "rh1" else rf(j)
                            nc.tensor.matmul(cps[ot][:], wf(GO + ot * 128), rhs,
                                             start=(ki == 0), stop=(ki == nk - 1))
                            ki += 1
                    for kt in range(2):
                        cc = cpp.tile([128, MC], F32, tag="c", name=f"c{kt}")
                        nc.scalar.activation(cc[:], cps[kt][:], AF.Tanh,
                                             bias=biases[:, bb + 4 + kt:bb + 5 + kt], scale=1.0)
                        # h' = c + u * (h - c)
                        d = tmpp.tile([128, MC], F32, tag="tmp", name=f"d{kt}")
                        nc.vector.tensor_sub(d[:], ht[kt][:, ms].bitcast(F32), cc[:])
                        e = tmpp.tile([128, MC], F32, tag="tmp", name=f"e{kt}")
                        nc.vector.tensor_mul(e[:], ru[2 + kt][:], d[:])
                        nc.vector.tensor_add(ht[kt][:, ms], cc[:], e[:])

            # ================= encoder =================
            for t in range(S):
                # layer 0: x part via masked weights (K=24), then h (K=256)
                gk0 = [(lambda ob, t=t: emask[:, t, bass.ds(ob, 128)],
                        lambda j: xct[:, bass.ds(j * MC, MC)]),
                       (lambda ob: enc0h[:, 0, bass.ds(ob, 128)],
                        lambda j: h00[:, bass.ds(j * MC, MC)]),
                       (lambda ob: enc0h[:, 1, bass.ds(ob, 128)],
                        lambda j: h01[:, bass.ds(j * MC, MC)])]
                ck0 = [(lambda ob, t=t: emask[:, t, bass.ds(ob, 128)],
                        lambda j: xct[:, bass.ds(j * MC, MC)]),
                       (lambda ob: enc0h[:, 0, bass.ds(ob, 128)], "rh0"),
                       (lambda ob: enc0h[:, 1, bass.ds(ob, 128)], "rh1")]
                gru_layer(gk0, ck0, h0, 0)
                # layer 1: x part = h0 (new), own h = h1
                gk1 = [(lambda ob: enc1w[:, 2, bass.ds(ob, 128)],
                        lambda j: h10[:, bass.ds(j * MC, MC)]),
                       (lambda ob: enc1w[:, 3, bass.ds(ob, 128)],
                        lambda j: h11[:, bass.ds(j * MC, MC)]),
                       (lambda ob: enc1w[:, 0, bass.ds(ob, 128)],
                        lambda j: h00[:, bass.ds(j * MC, MC)]),
                       (lambda ob: enc1w[:, 1, bass.ds(ob, 128)],
                        lambda j: h01[:, bass.ds(j * MC, MC)])]
                ck1 = [(lambda ob: enc1w[:, 0, bass.ds(ob, 128)],
                        lambda j: h00[:, bass.ds(j * MC, MC)]),
                       (lambda ob: enc1w[:, 1, bass.ds(ob, 128)],
                        lambda j: h01[:, bass.ds(j * MC, MC)]),
                       (lambda ob: enc1w[:, 2, bass.ds(ob, 128)], "rh0"),
                       (lambda ob: enc1w[:, 3, bass.ds(ob, 128)], "rh1")]
                gru_layer(gk1, ck1, h1, 6)

            # ================= decoder =================
            dmask = maskp.tile([S, S, TO], F32R, tag="mask", name="dmask")
            nc.sync.dma_start(dmask[:], dec0m_d.rearrange("p (t o) -> p t o", t=S))
            Y = xyp.tile([S + 1, M], F32R, tag="xy", name="Y")
            nc.sync.dma_start(Y[:], zeros_d[0:S + 1, :])
            nc.sync.dma_start(Y[0:1, :], y0_d)

            for t in range(horizon):
                gk0 = [(lambda ob: dec0h[:, 0, bass.ds(ob, 128)],
                        lambda j: h00[:, bass.ds(j * MC, MC)]),
                       (lambda ob: dec0h[:, 1, bass.ds(ob, 128)],
                        lambda j: h01[:, bass.ds(j * MC, MC)]),
                       (lambda ob, t=t: dmask[:, t, bass.ds(ob, 128)],
                        lambda j: Y[0:S, bass.ds(j * MC, MC)])]
                ck0 = [(lambda ob, t=t: dmask[:, t, bass.ds(ob, 128)],
                        lambda j: Y[0:S, bass.ds(j * MC, MC)]),
                       (lambda ob: dec0h[:, 0, bass.ds(ob, 128)], "rh0"),
                       (lambda ob: dec0h[:, 1, bass.ds(ob, 128)], "rh1")]
                gru_layer(gk0, ck0, h0, 12)
                gk1 = [(lambda ob: dec1w[:, 2, bass.ds(ob, 128)],
                        lambda j: h10[:, bass.ds(j * MC, MC)]),
                       (lambda ob: dec1w[:, 3, bass.ds(ob, 128)],
                        lambda j: h11[:, bass.ds(j * MC, MC)]),
                       (lambda ob: dec1w[:, 0, bass.ds(ob, 128)],
                        lambda j: h00[:, bass.ds(j * MC, MC)]),
                       (lambda ob: dec1w[:, 1, bass.ds(ob, 128)],
                        lambda j: h01[:, bass.ds(j * MC, MC)])]
                ck1 = [(lambda ob: dec1w[:, 0, bass.ds(ob, 128)],
                        lambda j: h00[:, bass.ds(j * MC, MC)]),
                       (lambda ob: dec1w[:, 1, bass.ds(ob, 128)],
                        lambda j: h01[:, bass.ds(j * MC, MC)]),
                       (lambda ob: dec1w[:, 2, bass.ds(ob, 128)], "rh0"),
                       (lambda ob: dec1w[:, 3, bass.ds(ob, 128)], "rh1")]
                gru_layer(gk1, ck1, h1, 18)
                # proj: y_{t+1} = Wp @ h1 -> stage row (partition 0), then
                # DMA into Y row t+1 (compute engines can't write partition>0)
                ystage = ystage_pool.tile([1, M], F32R, tag="ys", name="ystage")
                for j in range(NJ):
                    ms = bass.ds(j * MC, MC)
                    pp = pjp.tile([1, MC], F32, tag="pj", name="pp")
                    nc.tensor.matmul(pp[:], wproj[:, 0:1], h10[:, ms],
                                     start=True, stop=False)
                    nc.tensor.matmul(pp[:], wproj[:, 1:2], h11[:, ms],
                                     start=False, stop=True)
                    nc.vector.tensor_copy(ystage[0:1, ms], pp[:])
                nc.sync.dma_start(Y[t + 1:t + 2, :], ystage[0:1, :])

            nc.sync.dma_start(y_d, Y[1:horizon + 1, :])

    nc.compile()
    return nc


def _prep_host(x, I, params, horizon):
    """Host-side: downscale pooling + weight packing. Returns in_maps."""
    DI = I / np.sum(np.abs(I), axis=1, keepdims=True)          # [C, N]
    # xc[b,f,s,c] = sum_n x[b,f,n,s] * DI[c,n]
    xr = np.ascontiguousarray(x.transpose(0, 1, 3, 2)).reshape(-1, N)  # [(b f s), N]
    xc = (xr @ DI.T).reshape(B, F_IN, S, C)                    # [B, F, S, C]

    enc, dec = params["enc"], params["dec"]

    def pack_l0(p, fin):
        Wg, Wc = np.asarray(p["Wg"], np.float32), np.asarray(p["Wc"], np.float32)
        Wx = np.concatenate([Wg[:, :fin], Wc[:, :fin]], axis=0)     # [TO, fin]
        Wh = np.concatenate([Wg[:, fin:], Wc[:, fin:]], axis=0)     # [TO, H]
        return Wx, np.ascontiguousarray(Wh.T)                       # WhT [H, TO]

    def pack_l1(p):
        Wg, Wc = np.asarray(p["Wg"], np.float32), np.asarray(p["Wc"], np.float32)
        Wall = np.concatenate([Wg, Wc], axis=0)                     # [TO, 2H]
        return np.ascontiguousarray(Wall.T)                         # [2H, TO]

    e0x, e0h = pack_l0(enc[0], F_IN)
    d0x, d0h = pack_l0(dec[0], OUT)
    e1 = pack_l1(enc[1])
    d1 = pack_l1(dec[1])

    enc0m = np.zeros((2 * S, S, TO), np.float32)
    for f in range(F_IN):
        enc0m[np.arange(S) * 2 + f, np.arange(S), :] = e0x[:, f]
    dec0m = np.zeros((S, S, TO), np.float32)
    dec0m[np.arange(S), np.arange(S), :] = d0x[:, 0]

    def kfold(WT, nk):    # [K, TO] -> [128, nk*TO]
        return np.ascontiguousarray(WT.reshape(nk, 128, TO).transpose(1, 0, 2)
                                    ).reshape(128, nk * TO)

    bp = float(np.asarray(params["proj_b"], np.float32).reshape(-1)[0])
    bias_np = np.zeros((128, 24), np.float32)
    for li, p in enumerate([enc[0], enc[1], dec[0], dec[1]]):
        bg = np.asarray(p["bg"], np.float32).copy()
        bc = np.asarray(p["bc"], np.float32).copy()
        if li == 2:
            # decoder L0 consumes un-biased y; fold Wy*bp into its biases
            bg = bg + d0x[:GO, 0] * bp
            bc = bc + d0x[GO:, 0] * bp
        for ot in range(4):
            bias_np[:, li * 6 + ot] = bg[ot * 128:(ot + 1) * 128]
        for kt in range(2):
            bias_np[:, li * 6 + 4 + kt] = bc[kt * 128:(kt + 1) * 128]

    shared = {
        "enc0m": np.ascontiguousarray(enc0m.reshape(2 * S, S * TO)),
        "enc0h": kfold(e0h, 2),
        "enc1": kfold(e1, 4),
        "dec0m": np.ascontiguousarray(dec0m.reshape(S, S * TO)),
        "dec0h": kfold(d0h, 2),
        "dec1": kfold(d1, 4),
        "wproj": np.ascontiguousarray(
            np.asarray(params["proj_W"], np.float32).reshape(H).reshape(2, 128).T),
        "bias": bias_np,
    }
    in_maps = []
    zeros = np.zeros((128, M), np.float32)
    for k in range(NCORES):
        xk = xc[k * BL:(k + 1) * BL]                       # [BL, F, S, C]
        xct = np.ascontiguousarray(xk.transpose(2, 1, 0, 3)).reshape(2 * S, M)
        m = dict(shared)
        m["zeros"] = zeros
        m["xct"] = xct
        # y0 = xc[:, 0, :, -1] (f=0, s=S-1), minus the proj bias offset
        m["y0"] = np.ascontiguousarray(xct[2 * (S - 1):2 * (S - 1) + 1, :] - bp)
        in_maps.append(m)
    return in_maps, I, bp


def kernel(x, I, params, horizon):
    global LAST_EXEC_NS
    x = np.asarray(x, np.float32)
    I = np.asarray(I, np.float32)
    horizon = int(horizon)
    assert x.shape == (B, F_IN, N, S) and I.shape == (C, N)
    if horizon <= 0:
        return np.zeros((B, OUT, N, 0), np.float32)
    assert horizon <= S

    in_maps, I_full, bp = _prep_host(x, I, params, horizon)

    if horizon not in _PROG_CACHE:
        _PROG_CACHE[horizon] = _build_program(horizon)
    nc = _PROG_CACHE[horizon]

    import time as _time
    _t0 = _time.perf_counter()
    res = bass_utils.run_bass_kernel_spmd(
        nc, in_maps, core_ids=list(range(NCORES)))
    LAST_EXEC_NS = int((_time.perf_counter() - _t0) * 1e9)

    # gather + upscale on host
    yc = np.empty((B, C, horizon), np.float32)
    for k in range(NCORES):
        yk = res.results[k]["y"] + bp             # [horizon, M]; re-add proj bias
        yc[k * BL:(k + 1) * BL] = yk.reshape(horizon, BL, C).transpose(1, 2, 0)
    up = (np.ascontiguousarray(yc.transpose(0, 2, 1)).reshape(-1, C) @ I_full)
    out = up.reshape(B, horizon, N).transpose(0, 2, 1).reshape(B, OUT, N, horizon)
    return np.ascontiguousarray(out)
